# revision 10
# baseline (speedup 1.0000x reference)
"""BiGCN (bidirectional 2-layer GCN over many small graphs) on 8 Trainium2 cores.

Strategy: data-parallel over graphs (32 graphs of 128 nodes per core). Each
graph's GCN aggregation is a dense 128x128 matmul with the normalized
adjacency, built on-device from the edge lists via one-hot matmuls.

Math (per branch, per graph, n=128 nodes):
  A[d,s]   = #edges s->d (incl. self loops)          -- built as OneHotSrc^T-style matmul
  deg      = row-sums of A;  norm = 1/sqrt(deg)
  h   = relu(norm * (A @ (norm * (X @ W1))) + b1)
  Z   = h @ W2h + ones ⊗ (x_root @ W2r)              -- rank-1 fold of the root term
  H2  = relu(norm * (A @ (norm * Z)) + b2)
  out = [mean_nodes(H2), h[root]]                     -- per-graph readout [512]
Final output: concat(TD branch, BU branch) -> [G, 1024].
"""

import numpy as np

import concourse.bass as bass
import concourse.tile as tile
from concourse import bacc, mybir
from concourse.bass_utils import run_bass_kernel_spmd
from concourse.masks import make_identity

# Problem shape (fixed by the task)
N_GRAPHS = 256
N_PER_G = 128
IN_FEATS = 768
H_FEATS = 256
N_CORES = 8
G_PER_CORE = N_GRAPHS // N_CORES            # 32
NODES_PER_CORE = G_PER_CORE * N_PER_G       # 4096
KCH = IN_FEATS // 128                       # 6 feature chunks

MM_DT = mybir.dt.float32r                   # dtype of SBUF matmul operands
OH_DT = mybir.dt.bfloat16                   # one-hot tiles (0/1, exact)
F32 = mybir.dt.float32
I32 = mybir.dt.int32
AF = mybir.ActivationFunctionType
OP = mybir.AluOpType


# ----------------------------------------------------------------------------
# Host-side edge packing
# ----------------------------------------------------------------------------

def pack_edges(src, dst, n, G):
    """Group intra-graph edges by graph; returns local-index arrays
    [G, C*128] padded with -1, plus identity_flag (self-loops folded as +I)
    and chunk count C."""
    src = np.asarray(src, np.int64)
    dst = np.asarray(dst, np.int64)
    g = dst // n
    if not np.array_equal(src // n, g):
        raise ValueError("cross-graph edge found; contiguous-block sharding invalid")
    self_mask = src == dst
    identity_flag = False
    if int(self_mask.sum()) == G * n:
        sl = np.sort(src[self_mask])
        if np.array_equal(sl, np.arange(G * n)):
            identity_flag = True
            keep = ~self_mask
            src, dst, g = src[keep], dst[keep], g[keep]
    order = np.argsort(g, kind="stable")
    src, dst, g = src[order], dst[order], g[order]
    counts = np.bincount(g, minlength=G)
    cmax = int(counts.max()) if len(counts) else 0
    C = max(1, -(-cmax // 128))
    src_p = np.full((G, C * 128), -1.0, np.float32)
    dst_p = np.full((G, C * 128), -1.0, np.float32)
    starts = np.concatenate([[0], np.cumsum(counts)])
    for gi in range(G):
        s, e = int(starts[gi]), int(starts[gi + 1])
        src_p[gi, : e - s] = src[s:e] - gi * n
        dst_p[gi, : e - s] = dst[s:e] - gi * n
    return src_p, dst_p, identity_flag, C


# ----------------------------------------------------------------------------
# Device program (SPMD; one core's shard)
# ----------------------------------------------------------------------------

def build_program(C_td, C_bu, ident_td, ident_bu, has_bias):
    nc = bacc.Bacc("TRN2", target_bir_lowering=False, debug=False,
                   num_devices=N_CORES)

    def din(name, shape, dt=F32):
        return nc.dram_tensor(name, shape, dt, kind="ExternalInput").ap()

    xt = din("xt", [IN_FEATS, NODES_PER_CORE], MM_DT)
    xrootst = din("xrootst", [IN_FEATS, G_PER_CORE], MM_DT)
    w1p = din("w1p", [IN_FEATS, 2 * H_FEATS], MM_DT)
    w2h_td = din("w2h_td", [H_FEATS, H_FEATS], MM_DT)
    w2h_bu = din("w2h_bu", [H_FEATS, H_FEATS], MM_DT)
    w2rp = din("w2rp", [IN_FEATS, 2 * H_FEATS], MM_DT)
    src_td = din("src_td", [128, G_PER_CORE * C_td])
    dst_td = din("dst_td", [128, G_PER_CORE * C_td])
    src_bu = din("src_bu", [128, G_PER_CORE * C_bu])
    dst_bu = din("dst_bu", [128, G_PER_CORE * C_bu])
    if has_bias:
        b1_td = din("b1_td", [128, H_FEATS])
        b2_td = din("b2_td", [128, H_FEATS])
        b1_bu = din("b1_bu", [128, H_FEATS])
        b2_bu = din("b2_bu", [128, H_FEATS])
    out = nc.dram_tensor("out", [G_PER_CORE, 4 * H_FEATS], F32,
                         kind="ExternalOutput").ap()

    CC = {0: C_td, 1: C_bu}
    IDENT = {0: ident_td, 1: ident_bu}
    SRC = {0: src_td, 1: src_bu}
    DST = {0: dst_td, 1: dst_bu}
    W2H = {0: w2h_td, 1: w2h_bu}

    with tile.TileContext(nc) as tc:
        with (
            tc.tile_pool(name="const", bufs=1) as const,
            tc.tile_pool(name="xin", bufs=3) as xin,
            tc.tile_pool(name="oh", bufs=3) as ohp,
            tc.tile_pool(name="adj", bufs=3) as adjp,
            tc.tile_pool(name="act", bufs=3) as actp,
            tc.tile_pool(name="ps256", bufs=2, space="PSUM") as ps256,
            tc.tile_pool(name="ps128", bufs=2, space="PSUM") as ps128,
            tc.tile_pool(name="psY", bufs=1, space="PSUM") as psY,
            tc.tile_pool(name="psD", bufs=1, space="PSUM") as psD,
            tc.tile_pool(name="psS", bufs=1, space="PSUM") as psS,
        ):
            # ---- constants -------------------------------------------------
            iota_t = const.tile([128, 128], F32)
            nc.gpsimd.iota(iota_t[:], pattern=[[1, 128]], base=0,
                           channel_multiplier=0,
                           allow_small_or_imprecise_dtypes=True)
            # walrus rejects gpsimd memset/affine_select on float32r tiles, so
            # build f32 versions and cast-copy on the vector engine
            identity_f32 = const.tile([128, 128], F32)
            make_identity(nc, identity_f32[:])
            identity = const.tile([128, 128], MM_DT)
            nc.vector.tensor_copy(identity[:], identity_f32[:])
            ones_col = const.tile([128, 1], OH_DT)
            nc.gpsimd.memset(ones_col[:], 1.0)
            ones_row_f32 = const.tile([1, 128], F32)
            nc.gpsimd.memset(ones_row_f32[:], 1.0)
            ones_row = const.tile([1, 128], MM_DT)
            nc.vector.tensor_copy(ones_row[:], ones_row_f32[:])
            mean_col_f32 = const.tile([128, 1], F32)
            nc.gpsimd.memset(mean_col_f32[:], 1.0 / N_PER_G)
            mean_col = const.tile([128, 1], MM_DT)
            nc.vector.tensor_copy(mean_col[:], mean_col_f32[:])

            w1p_sb = const.tile([128, KCH, 2 * H_FEATS], MM_DT)
            nc.sync.dma_start(w1p_sb[:], w1p.rearrange("(ko p) n -> p ko n", p=128))
            w2rp_sb = const.tile([128, KCH, 2 * H_FEATS], MM_DT)
            nc.sync.dma_start(w2rp_sb[:], w2rp.rearrange("(ko p) n -> p ko n", p=128))
            xrootst_sb = const.tile([128, KCH, G_PER_CORE], MM_DT)
            nc.sync.dma_start(xrootst_sb[:],
                              xrootst.rearrange("(ko p) n -> p ko n", p=128))
            w2h_sb = {}
            for b in (0, 1):
                t = const.tile([128, 2, H_FEATS], MM_DT, tag=f"w2h_{b}")
                nc.sync.dma_start(t[:], W2H[b].rearrange("(ko p) n -> p ko n", p=128))
                w2h_sb[b] = t
            srcp_sb, dstp_sb = {}, {}
            for b in (0, 1):
                s = const.tile([128, G_PER_CORE * CC[b]], F32, tag=f"srcp_{b}")
                nc.sync.dma_start(s[:], SRC[b])
                d = const.tile([128, G_PER_CORE * CC[b]], F32, tag=f"dstp_{b}")
                nc.sync.dma_start(d[:], DST[b])
                srcp_sb[b], dstp_sb[b] = s, d
            bias_sb = {}
            if has_bias:
                for nm, ap_ in (("b1_td", b1_td), ("b2_td", b2_td),
                                ("b1_bu", b1_bu), ("b2_bu", b2_bu)):
                    t = const.tile([128, H_FEATS], F32, tag=nm)
                    nc.sync.dma_start(t[:], ap_)
                    bias_sb[nm] = t

            # ---- rvec = Xroots @ [W2r_td | W2r_bu]  -> [32, 512] -----------
            ps_rv = psS.tile([G_PER_CORE, 2 * H_FEATS], F32, tag="ps_small")
            for k in range(KCH):
                nc.tensor.matmul(ps_rv[:], xrootst_sb[:, k, :], w2rp_sb[:, k, :],
                                 start=(k == 0), stop=(k == KCH - 1))
            rvec_sb = const.tile([G_PER_CORE, 2 * H_FEATS], MM_DT)
            nc.vector.tensor_copy(rvec_sb[:], ps_rv[:])
            # flatten to one partition so per-graph rows can be matmul rhs
            # (matmul operands must start at base partition 0); SBUF partitions
            # can't be flattened into the free dim directly, so bounce via DRAM
            rv_dram = nc.dram_tensor("rv_scratch", [G_PER_CORE, 2 * H_FEATS],
                                     MM_DT).ap()
            nc.sync.dma_start(rv_dram[:], rvec_sb[:])
            rvec_row = const.tile([1, G_PER_CORE * 2 * H_FEATS], MM_DT)
            nc.sync.dma_start(rvec_row[:],
                              rv_dram.rearrange("g f -> (g f)")[None, :])

            xt_re = xt.rearrange("(ko p) n -> p ko n", p=128)

            # ---- main loop over this core's graphs -------------------------
            for g in range(G_PER_CORE):
                xt_tile = xin.tile([128, KCH, 128], MM_DT, tag="xt_tile")
                nc.sync.dma_start(xt_tile[:], xt_re[:, :, g * 128:(g + 1) * 128])

                at_sb, norm = {}, {}
                for b in (0, 1):
                    Cb = CC[b]
                    oh_s = ohp.tile([128, Cb * 128], OH_DT, tag="oh_s")
                    oh_d = ohp.tile([128, Cb * 128], OH_DT, tag="oh_d")
                    for c in range(Cb):
                        j = g * Cb + c
                        nc.vector.tensor_scalar(
                            oh_s[:, c * 128:(c + 1) * 128], iota_t[:],
                            srcp_sb[b][:, j:j + 1], None, OP.is_equal)
                        nc.vector.tensor_scalar(
                            oh_d[:, c * 128:(c + 1) * 128], iota_t[:],
                            dstp_sb[b][:, j:j + 1], None, OP.is_equal)
                    ps_a = ps128.tile([128, 128], F32, tag="ps128")
                    ps_d = psD.tile([128, 1], F32, tag="psD")
                    for c in range(Cb):
                        cs = slice(c * 128, (c + 1) * 128)
                        nc.tensor.matmul(ps_a[:], oh_s[:, cs], oh_d[:, cs],
                                         start=(c == 0), stop=(c == Cb - 1))
                        nc.tensor.matmul(ps_d[:], oh_d[:, cs], ones_col[:],
                                         start=(c == 0), stop=(c == Cb - 1))
                    at = adjp.tile([128, 128], MM_DT, tag="at")
                    if IDENT[b]:
                        nc.vector.tensor_add(at[:], ps_a[:], identity[:])
                    else:
                        nc.vector.tensor_copy(at[:], ps_a[:])
                    sq = adjp.tile([128, 1], F32, tag="sq")
                    nc.scalar.activation(sq[:], ps_d[:], AF.Sqrt,
                                         bias=1.0 if IDENT[b] else 0.0)
                    nm = adjp.tile([128, 1], F32, tag="nm")
                    nc.vector.reciprocal(nm[:], sq[:])
                    at_sb[b], norm[b] = at, nm

                # Y = X @ [W1_td | W1_bu]
                ps_y = psY.tile([128, 2 * H_FEATS], F32, tag="psY")
                for k in range(KCH):
                    nc.tensor.matmul(ps_y[:], xt_tile[:, k, :], w1p_sb[:, k, :],
                                     start=(k == 0), stop=(k == KCH - 1))

                for b in (0, 1):
                    col = slice(b * H_FEATS, (b + 1) * H_FEATS)
                    nm = norm[b]
                    bname1 = "b1_td" if b == 0 else "b1_bu"
                    bname2 = "b2_td" if b == 0 else "b2_bu"

                    yn = actp.tile([128, H_FEATS], MM_DT, tag="yn")
                    nc.vector.tensor_scalar(yn[:], ps_y[:, col], nm[:], None,
                                            OP.mult)
                    ps1 = ps256.tile([128, H_FEATS], F32, tag="ps256")
                    nc.tensor.matmul(ps1[:], at_sb[b][:], yn[:])
                    h = actp.tile([128, H_FEATS], MM_DT, tag="h")
                    if has_bias:
                        nc.vector.tensor_scalar(h[:], ps1[:], nm[:], None, OP.mult)
                        nc.vector.tensor_add(h[:], h[:], bias_sb[bname1][:])
                        nc.vector.tensor_scalar(h[:], h[:], 0.0, None, OP.max)
                    else:
                        nc.vector.tensor_scalar(h[:], ps1[:], nm[:], 0.0,
                                                OP.mult, OP.max)
                    # readout part 2: h at root node (DMA: compute engines
                    # cannot write at arbitrary start partitions)
                    nc.gpsimd.dma_start(
                        out[g:g + 1, b * 512 + 256: b * 512 + 512], h[0:1, :])

                    # hT via PE transpose
                    ht = actp.tile([128, 2, 128], MM_DT, tag="ht")
                    for j in (0, 1):
                        ps_t = ps128.tile([128, 128], MM_DT, tag="ps128")
                        nc.tensor.transpose(ps_t[:], h[:, j * 128:(j + 1) * 128],
                                            identity[:])
                        nc.scalar.copy(ht[:, j, :], ps_t[:])

                    # Z = h @ W2h + ones ⊗ rvec
                    ps_z = ps256.tile([128, H_FEATS], F32, tag="ps256")
                    nc.tensor.matmul(ps_z[:], ht[:, 0, :], w2h_sb[b][:, 0, :],
                                     start=True, stop=False)
                    nc.tensor.matmul(ps_z[:], ht[:, 1, :], w2h_sb[b][:, 1, :],
                                     start=False, stop=False)
                    rv_off = g * 2 * H_FEATS + b * H_FEATS
                    nc.tensor.matmul(ps_z[:], ones_row[:],
                                     rvec_row[0:1, rv_off:rv_off + H_FEATS],
                                     start=False, stop=True)
                    zn = actp.tile([128, H_FEATS], MM_DT, tag="zn")
                    nc.vector.tensor_scalar(zn[:], ps_z[:], nm[:], None, OP.mult)
                    ps2 = ps256.tile([128, H_FEATS], F32, tag="ps256")
                    nc.tensor.matmul(ps2[:], at_sb[b][:], zn[:])
                    h2 = actp.tile([128, H_FEATS], MM_DT, tag="h2")
                    if has_bias:
                        nc.vector.tensor_scalar(h2[:], ps2[:], nm[:], None, OP.mult)
                        nc.vector.tensor_add(h2[:], h2[:], bias_sb[bname2][:])
                        nc.vector.tensor_scalar(h2[:], h2[:], 0.0, None, OP.max)
                    else:
                        nc.vector.tensor_scalar(h2[:], ps2[:], nm[:], 0.0,
                                                OP.mult, OP.max)

                    # readout part 1: mean over nodes
                    ps_m = psS.tile([1, H_FEATS], F32, tag="ps_small")
                    nc.tensor.matmul(ps_m[:], mean_col[:], h2[:])
                    mrow = actp.tile([1, H_FEATS], F32, tag="mrow")
                    nc.scalar.copy(mrow[:], ps_m[:])
                    nc.sync.dma_start(out[g:g + 1, b * 512: b * 512 + 256],
                                      mrow[:])

    nc.compile()
    return nc


# ----------------------------------------------------------------------------
# Host entry point
# ----------------------------------------------------------------------------

def _prep(inputs, w1_td, b1_td, w2_td, b2_td, w1_bu, b1_bu, w2_bu, b2_bu,
          td_src, td_dst, bu_src, bu_dst, nodes_per_graph):
    n = int(nodes_per_graph)
    X = np.ascontiguousarray(np.asarray(inputs, np.float32))
    N = X.shape[0]
    G = N // n
    assert (n, G, X.shape[1]) == (N_PER_G, N_GRAPHS, IN_FEATS), \
        f"unexpected shapes {X.shape} n={n}"

    sp_td, dp_td, id_td, C_td = pack_edges(td_src, td_dst, n, G)
    sp_bu, dp_bu, id_bu, C_bu = pack_edges(bu_src, bu_dst, n, G)

    w1p = np.ascontiguousarray(
        np.concatenate([np.asarray(w1_td, np.float32),
                        np.asarray(w1_bu, np.float32)], axis=1))
    w2_td = np.asarray(w2_td, np.float32)
    w2_bu = np.asarray(w2_bu, np.float32)
    w2rp = np.ascontiguousarray(
        np.concatenate([w2_td[H_FEATS:], w2_bu[H_FEATS:]], axis=1))
    biases = [np.asarray(b, np.float32) for b in (b1_td, b2_td, b1_bu, b2_bu)]
    has_bias = any(np.any(b != 0) for b in biases)

    in_maps = []
    for c in range(N_CORES):
        gs = slice(c * G_PER_CORE, (c + 1) * G_PER_CORE)
        ns = slice(c * NODES_PER_CORE, (c + 1) * NODES_PER_CORE)
        Xc = X[ns]
        m = {
            "xt": np.ascontiguousarray(Xc.T),
            "xrootst": np.ascontiguousarray(Xc[::n].T),
            "w1p": w1p,
            "w2h_td": np.ascontiguousarray(w2_td[:H_FEATS]),
            "w2h_bu": np.ascontiguousarray(w2_bu[:H_FEATS]),
            "w2rp": w2rp,
            "src_td": np.ascontiguousarray(
                sp_td[gs].reshape(G_PER_CORE * C_td, 128).T),
            "dst_td": np.ascontiguousarray(
                dp_td[gs].reshape(G_PER_CORE * C_td, 128).T),
            "src_bu": np.ascontiguousarray(
                sp_bu[gs].reshape(G_PER_CORE * C_bu, 128).T),
            "dst_bu": np.ascontiguousarray(
                dp_bu[gs].reshape(G_PER_CORE * C_bu, 128).T),
        }
        if has_bias:
            m["b1_td"] = np.ascontiguousarray(
                np.broadcast_to(biases[0], (128, H_FEATS)))
            m["b2_td"] = np.ascontiguousarray(
                np.broadcast_to(biases[1], (128, H_FEATS)))
            m["b1_bu"] = np.ascontiguousarray(
                np.broadcast_to(biases[2], (128, H_FEATS)))
            m["b2_bu"] = np.ascontiguousarray(
                np.broadcast_to(biases[3], (128, H_FEATS)))
        in_maps.append(m)
    return in_maps, (C_td, C_bu, id_td, id_bu, has_bias)


_PROGRAM_CACHE = {}


def _get_program(key):
    if key not in _PROGRAM_CACHE:
        _PROGRAM_CACHE[key] = build_program(*key)
    return _PROGRAM_CACHE[key]


def kernel(trace=False, tmpdir=None, _return_raw=False, **inputs):
    in_maps, key = _prep(**inputs)
    nc = _get_program(key)
    res = run_bass_kernel_spmd(nc, in_maps, list(range(N_CORES)),
                               trace=trace, tmpdir=tmpdir)
    out = np.concatenate([res.results[i]["out"] for i in range(N_CORES)], axis=0)
    if _return_raw:
        return out, res
    return out
